# revision 11
# baseline (speedup 1.0000x reference)
"""MinGRU forward on 8 TRN2 NeuronCores.

Math (linear-space reformulation of the reference's log-space Heinsen scan):
    hg = x @ W_hg.T                       # [B,S,2D]
    hidden, gate = split(hg)
    z = sigmoid(gate)
    c = sigmoid(-gate)                    # = 1 - z
    g = max(hidden + 0.5, sigmoid(hidden))  # == where(h>=0, h+0.5, sigmoid(h))
    u = z * g
    h[t] = c[t] * h[t-1] + u[t]           # convex combination -> bounded, stable
    out = h

Sharding: 8 cores = 4 batches x 2 feature-halves (512 features each).
No cross-core communication: the scan is per-feature independent.

All HBM I/O and matmul operands are fp16 (11 mantissa bits ~ fp32r's
host-rounded 11; verified max rel err ~2.4e-3 vs the 2e-2 gate) which halves
DMA traffic, SBUF footprint and LDWEIGHTS time vs fp32r at the same PE rate.
PSUM accumulation stays fp32; the scan's internal state is fp32.

Engine split per [128, width] tile:
  ACT:  st = sigmoid(ph), ct = sigmoid(-pg)            (2 ops, psum -> fp16)
  DVE:  gt = (ph + 0.5) max st ; zt = 1 - ct ; ut = zt*gt ; scan(ct, ut)
  PE:   16 fp16 matmuls (8 ph + 8 pg), psum fp32
  ACT ring: W loads (fc0 k-sliced for a fast first matmul)
  SP ring:  x in (chunk 0 k-sliced), h out (batched per chunk)
"""

import numpy as np

B, S, D = 4, 4096, 1024
DH = D // 2          # features per core
N_CORES = 8
KC = 128             # contraction chunk
NKC = D // KC        # 8 k chunks
FC = 128             # feature chunk (psum partitions)
NFC = DH // FC       # 4 feature chunks
WIDTHS = [512, 512, 512, 512, 512, 512, 512, 256, 256]
assert sum(WIDTHS) == S

_CACHE = {}


def _build():
    import concourse.bacc as bacc
    import concourse.tile as tile
    import concourse.mybir as mybir

    f32 = mybir.dt.float32
    f16 = mybir.dt.float16
    AF = mybir.ActivationFunctionType
    OP = mybir.AluOpType

    nc = bacc.Bacc("TRN2")
    xT = nc.dram_tensor("xT", [D, S], f16, kind="ExternalInput")
    # wT layout: [D, NFC, 2*FC] — per feature-chunk fc, 128 hidden cols then
    # 128 gate cols.
    wT = nc.dram_tensor("wT", [D, NFC, 2 * FC], f16, kind="ExternalInput")
    outT = nc.dram_tensor("outT", [DH, S], f16, kind="ExternalOutput")

    with tile.TileContext(nc) as tc:
        with (
            tc.tile_pool(name="w", bufs=1) as wpool,
            tc.tile_pool(name="x", bufs=3) as xpool,
            tc.tile_pool(name="ew", bufs=3) as epool,
            tc.tile_pool(name="h", bufs=2) as hpool,
            tc.tile_pool(name="ps", bufs=4, space="PSUM") as pspool,
        ):
            # All W rides the ACT ring; x and out ride the SP ring. fc0 is
            # k-sliced (finest first) so the very first matmul only waits on
            # 64KB, and the fc-major consumption order matches W's arrival
            # order.
            wts = []
            for fc in range(NFC):
                wtf = wpool.tile([KC, NKC, 2 * FC], f16, tag=f"w{fc}")
                wts.append(wtf)

            def load_w(fc, k0, k1):
                nc.scalar.dma_start(
                    wts[fc][:, k0:k1, :],
                    wT[k0 * KC:k1 * KC, fc, :].rearrange("(k p) e -> p k e", p=KC),
                )

            # fc0-k0 split h/g so the first matmul waits on only 32KB.  Few,
            # mostly-coarse issues per ring: each ring has only 8 completion
            # sems, and a 9th in-flight DMA stalls the issuing sequencer on
            # sem rotation.  fc2 rides the SP ring (after x chunk 0) so all
            # of W is resident by the time the PE clock ramp completes.
            nc.scalar.dma_start(wts[0][:, 0, 0:FC], wT[0:KC, 0, 0:FC])
            nc.scalar.dma_start(wts[0][:, 0, FC:2 * FC], wT[0:KC, 0, FC:2 * FC])
            for k0, k1 in [(1, 2), (2, 4), (4, 8)]:
                load_w(0, k0, k1)
            load_w(1, 0, 8)
            load_w(3, 0, 8)

            # x chunk 0 arrives in k-order to pace the first fc's matmuls.
            xt0 = xpool.tile([KC, NKC, WIDTHS[0]], f16, tag="xt")
            xT_r0 = xT[:, 0:WIDTHS[0]].rearrange("(k p) s -> p k s", p=KC)
            for k0, k1 in [(0, 1), (1, 2), (2, 4), (4, 6), (6, 8)]:
                nc.sync.dma_start(xt0[:, k0:k1, :], xT_r0[:, k0:k1, :])
            nc.sync.dma_start(
                wts[2][:], wT[:, 2, :].rearrange("(k p) e -> p k e", p=KC)
            )

            hprev = None
            off = 0
            for sc, width in enumerate(WIDTHS):
                last = sc == len(WIDTHS) - 1
                if sc == 0:
                    xt = xt0
                else:
                    xt = xpool.tile([KC, NKC, width], f16, tag="xt")
                    xT_r = xT[:, off:off + width].rearrange("(k p) s -> p k s", p=KC)
                    nc.sync.dma_start(xt[:], xT_r)
                hall = hpool.tile([FC, NFC, width], f16, tag="hall")
                for fc in range(NFC):
                    ph = pspool.tile([FC, width], f32, tag="ph")
                    pg = pspool.tile([FC, width], f32, tag="pg")
                    if sc == 0:
                        # Interleave ph/pg per k so each arriving W/x slice
                        # feeds 2 matmuls immediately — keeps the PE fed
                        # while W streams in.
                        for k in range(NKC):
                            nc.tensor.matmul(
                                ph[:], wts[fc][:, k, 0:FC], xt[:, k, :],
                                start=(k == 0), stop=(k == NKC - 1),
                            )
                            nc.tensor.matmul(
                                pg[:], wts[fc][:, k, FC:2 * FC], xt[:, k, :],
                                start=(k == 0), stop=(k == NKC - 1),
                            )
                    else:
                        for k in range(NKC):
                            nc.tensor.matmul(
                                ph[:], wts[fc][:, k, 0:FC], xt[:, k, :],
                                start=(k == 0), stop=(k == NKC - 1),
                            )
                        for k in range(NKC):
                            nc.tensor.matmul(
                                pg[:], wts[fc][:, k, FC:2 * FC], xt[:, k, :],
                                start=(k == 0), stop=(k == NKC - 1),
                            )
                    st = epool.tile([FC, width], f16, tag="s")
                    ct = epool.tile([FC, width], f16, tag="c")
                    gt = epool.tile([FC, width], f16, tag="g")
                    vt = epool.tile([FC, width], f16, tag="v")
                    # st first: it heads the DVE critical chain (s->g->v->scan)
                    nc.scalar.activation(st[:], ph[:], AF.Sigmoid)
                    nc.scalar.activation(ct[:], pg[:], AF.Sigmoid, scale=-1.0)
                    # g = (hidden + 0.5) max sigmoid(hidden)
                    nc.vector.scalar_tensor_tensor(
                        gt[:], ph[:], 0.5, st[:], op0=OP.add, op1=OP.max
                    )
                    # v = (c - 1)*g = -z*g; the scan subtracts it:
                    #   h = c*h_prev - v = c*h_prev + z*g
                    nc.vector.scalar_tensor_tensor(
                        vt[:], ct[:], 1.0, gt[:], op0=OP.subtract, op1=OP.mult
                    )
                    ho = hall[:, fc, :]
                    pw = WIDTHS[sc - 1]
                    init = 0.0 if sc == 0 else hprev[:, fc, pw - 1:pw]
                    if last and fc == NFC - 1:
                        # The very last feature chunk is the serial tail:
                        # split its scan so the first half's out-DMA overlaps
                        # the second half.
                        hw_ = width // 2
                        nc.vector.tensor_tensor_scan(
                            ho[:, 0:hw_], ct[:, 0:hw_], vt[:, 0:hw_], init,
                            op0=OP.mult, op1=OP.subtract,
                        )
                        nc.sync.dma_start(
                            outT[fc * FC:(fc + 1) * FC, off:off + hw_],
                            ho[:, 0:hw_],
                        )
                        nc.vector.tensor_tensor_scan(
                            ho[:, hw_:width], ct[:, hw_:width], vt[:, hw_:width],
                            hall[:, fc, hw_ - 1:hw_], op0=OP.mult, op1=OP.subtract,
                        )
                        nc.sync.dma_start(
                            outT[fc * FC:(fc + 1) * FC, off + hw_:off + width],
                            ho[:, hw_:width],
                        )
                    elif last:
                        nc.vector.tensor_tensor_scan(
                            ho[:], ct[:], vt[:], init, op0=OP.mult, op1=OP.subtract
                        )
                        nc.sync.dma_start(
                            outT[fc * FC:(fc + 1) * FC, off:off + width], ho[:]
                        )
                    else:
                        nc.vector.tensor_tensor_scan(
                            ho[:], ct[:], vt[:], init, op0=OP.mult, op1=OP.subtract
                        )
                if not last:
                    nc.sync.dma_start(
                        outT[:, off:off + width].rearrange(
                            "(f p) s -> p f s", p=FC
                        ),
                        hall[:],
                    )
                hprev = hall
                off += width

    nc.compile()
    return nc


def _prep_in_maps(x: np.ndarray, W_hg: np.ndarray):
    x = np.asarray(x, dtype=np.float32)
    W_hg = np.asarray(W_hg, dtype=np.float32)
    xTs = [np.ascontiguousarray(x[b].T).astype(np.float16) for b in range(B)]
    wTs = []
    for c in range(2):
        # [D, NFC, 2*FC]: per fc, 128 hidden cols then 128 gate cols
        wt = np.empty((D, NFC, 2 * FC), dtype=np.float32)
        for fc in range(NFC):
            rows_h = W_hg[c * DH + fc * FC:c * DH + (fc + 1) * FC]      # [FC, D]
            rows_g = W_hg[D + c * DH + fc * FC:D + c * DH + (fc + 1) * FC]
            wt[:, fc, 0:FC] = rows_h.T
            wt[:, fc, FC:2 * FC] = rows_g.T
        wTs.append(wt.astype(np.float16))
    return [{"xT": xTs[core // 2], "wT": wTs[core % 2]} for core in range(N_CORES)]


def _get_runner():
    """Build the Bass module once and cache a compiled jax callable for it.

    Mirrors bass2jax.run_bass_via_pjrt's multi-core path, but keeps the
    jitted/sharded executable so repeat kernel() calls skip re-tracing.
    """
    if "runner" in _CACHE:
        return _CACHE["runner"]

    import jax
    from jax.experimental.shard_map import shard_map
    from jax.sharding import Mesh, PartitionSpec
    from concourse import bass2jax

    if "nc" not in _CACHE:
        _CACHE["nc"] = _build()
    nc = _CACHE["nc"]
    bass2jax.install_neuronx_cc_hook()

    in_names = ["xT", "wT"]
    out_name = "outT"
    out_shape, out_dtype = (DH, S), np.float16
    partition_name = nc.partition_id_tensor.name if nc.partition_id_tensor else None

    def _body(xT, wT, zout):
        operands = [xT, wT, zout]
        if partition_name is not None:
            operands.append(bass2jax.partition_id_tensor())
        outs = bass2jax._bass_exec_p.bind(
            *operands,
            out_avals=(jax.core.ShapedArray(out_shape, out_dtype),),
            in_names=tuple(in_names + [out_name] + ([partition_name] if partition_name else [])),
            out_names=(out_name,),
            lowering_input_output_aliases=(),
            sim_require_finite=True,
            sim_require_nnan=True,
            nc=nc,
        )
        return tuple(outs)

    devices = jax.devices()[:N_CORES]
    mesh = Mesh(np.asarray(devices), ("core",))
    sharded = jax.jit(
        shard_map(
            _body, mesh=mesh,
            in_specs=(PartitionSpec("core"),) * 3,
            out_specs=(PartitionSpec("core"),),
            check_rep=False,
        ),
        donate_argnums=(2,),
        keep_unused=True,
    )

    def run(in_maps):
        concat_x = np.concatenate([m["xT"] for m in in_maps], axis=0)
        concat_w = np.concatenate([m["wT"] for m in in_maps], axis=0)
        zeros = np.zeros((N_CORES * DH, S), np.float16)
        (out_arr,) = sharded(concat_x, concat_w, zeros)
        return np.asarray(out_arr).reshape(N_CORES, DH, S)

    _CACHE["runner"] = run
    return run


def kernel(x: np.ndarray, W_hg: np.ndarray) -> np.ndarray:
    run = _get_runner()
    in_maps = _prep_in_maps(x, W_hg)
    outs = run(in_maps)

    out = np.empty((B, S, D), dtype=np.float32)
    for core in range(N_CORES):
        b, c = core // 2, core % 2
        out[b, :, c * DH:(c + 1) * DH] = outs[core].T.astype(np.float32)
    return out


# revision 13
# speedup vs baseline: 1.0361x; 1.0361x over previous
"""MinGRU forward on 8 TRN2 NeuronCores.

Math (linear-space reformulation of the reference's log-space Heinsen scan):
    hg = x @ W_hg.T                       # [B,S,2D]
    hidden, gate = split(hg)
    z = sigmoid(gate)
    c = sigmoid(-gate)                    # = 1 - z
    g = max(hidden + 0.5, sigmoid(hidden))  # == where(h>=0, h+0.5, sigmoid(h))
    u = z * g
    h[t] = c[t] * h[t-1] + u[t]           # convex combination -> bounded, stable
    out = h

Sharding: 8 cores = 4 batches x 2 feature-halves (512 features each).
No cross-core communication: the scan is per-feature independent.

All HBM I/O and matmul operands are fp16 (11 mantissa bits ~ fp32r's
host-rounded 11; verified max rel err ~2.4e-3 vs the 2e-2 gate) which halves
DMA traffic, SBUF footprint and LDWEIGHTS time vs fp32r at the same PE rate.
PSUM accumulation stays fp32; the scan's internal state is fp32.

Engine split per [128, width] tile:
  ACT:  st = sigmoid(ph), ct = sigmoid(-pg)            (2 ops, psum -> fp16)
  DVE:  gt = (ph + 0.5) max st ; zt = 1 - ct ; ut = zt*gt ; scan(ct, ut)
  PE:   16 fp16 matmuls (8 ph + 8 pg), psum fp32
  ACT ring: W loads (fc0 k-sliced for a fast first matmul)
  SP ring:  x in (chunk 0 k-sliced), h out (batched per chunk)
"""

import numpy as np

B, S, D = 4, 4096, 1024
DH = D // 2          # features per core
N_CORES = 8
KC = 128             # contraction chunk
NKC = D // KC        # 8 k chunks
FC = 128             # feature chunk (psum partitions)
NFC = DH // FC       # 4 feature chunks
WIDTHS = [512, 512, 512, 512, 512, 512, 512, 256, 256]
assert sum(WIDTHS) == S

_CACHE = {}


def _build():
    import concourse.bacc as bacc
    import concourse.tile as tile
    import concourse.mybir as mybir

    f32 = mybir.dt.float32
    f16 = mybir.dt.float16
    AF = mybir.ActivationFunctionType
    OP = mybir.AluOpType

    nc = bacc.Bacc("TRN2")
    xT = nc.dram_tensor("xT", [D, S], f16, kind="ExternalInput")
    # wT layout: [D, NFC, 2*FC] — per feature-chunk fc, 128 hidden cols then
    # 128 gate cols.
    wT = nc.dram_tensor("wT", [D, NFC, 2 * FC], f16, kind="ExternalInput")
    outT = nc.dram_tensor("outT", [DH, S], f16, kind="ExternalOutput")

    with tile.TileContext(nc) as tc:
        with (
            tc.tile_pool(name="w", bufs=1) as wpool,
            tc.tile_pool(name="x", bufs=3) as xpool,
            tc.tile_pool(name="ew", bufs=3) as epool,
            tc.tile_pool(name="h", bufs=2) as hpool,
            tc.tile_pool(name="ps", bufs=4, space="PSUM") as pspool,
        ):
            # All W rides the ACT ring; x and out ride the SP ring. fc0 is
            # k-sliced (finest first) so the very first matmul only waits on
            # 64KB, and the fc-major consumption order matches W's arrival
            # order.
            wts = []
            for fc in range(NFC):
                wtf = wpool.tile([KC, NKC, 2 * FC], f16, tag=f"w{fc}")
                wts.append(wtf)

            def load_w(fc, k0, k1):
                nc.scalar.dma_start(
                    wts[fc][:, k0:k1, :],
                    wT[k0 * KC:k1 * KC, fc, :].rearrange("(k p) e -> p k e", p=KC),
                )

            # fc0-k0 split h/g so the first matmul waits on only 32KB.  Few,
            # mostly-coarse issues per ring: each ring has only 8 completion
            # sems, and a 9th in-flight DMA stalls the issuing sequencer on
            # sem rotation.  fc2 rides the SP ring (after x chunk 0) so all
            # of W is resident by the time the PE clock ramp completes.
            nc.scalar.dma_start(wts[0][:, 0, 0:FC], wT[0:KC, 0, 0:FC])
            nc.scalar.dma_start(wts[0][:, 0, FC:2 * FC], wT[0:KC, 0, FC:2 * FC])
            for k0, k1 in [(1, 2), (2, 4), (4, 8)]:
                load_w(0, k0, k1)
            load_w(1, 0, 4)
            load_w(3, 0, 8)

            # x chunk 0 arrives in k-order to pace the first fc's matmuls.
            xt0 = xpool.tile([KC, NKC, WIDTHS[0]], f16, tag="xt")
            xT_r0 = xT[:, 0:WIDTHS[0]].rearrange("(k p) s -> p k s", p=KC)
            for k0, k1 in [(0, 1), (1, 2), (2, 4), (4, 6), (6, 8)]:
                nc.sync.dma_start(xt0[:, k0:k1, :], xT_r0[:, k0:k1, :])
            load_w_sync = lambda fc, k0, k1: nc.sync.dma_start(
                wts[fc][:, k0:k1, :],
                wT[k0 * KC:k1 * KC, fc, :].rearrange("(k p) e -> p k e", p=KC),
            )
            load_w_sync(1, 4, 8)
            load_w_sync(2, 0, 8)

            hprev = None
            off = 0
            for sc, width in enumerate(WIDTHS):
                last = sc == len(WIDTHS) - 1
                if sc == 0:
                    xt = xt0
                else:
                    xt = xpool.tile([KC, NKC, width], f16, tag="xt")
                    xT_r = xT[:, off:off + width].rearrange("(k p) s -> p k s", p=KC)
                    nc.sync.dma_start(xt[:], xT_r)
                hall = hpool.tile([FC, NFC, width], f16, tag="hall")
                for fc in range(NFC):
                    ph = pspool.tile([FC, width], f32, tag="ph")
                    pg = pspool.tile([FC, width], f32, tag="pg")
                    if sc == 0:
                        # Interleave ph/pg per k so each arriving W/x slice
                        # feeds 2 matmuls immediately — keeps the PE fed
                        # while W streams in.
                        for k in range(NKC):
                            nc.tensor.matmul(
                                ph[:], wts[fc][:, k, 0:FC], xt[:, k, :],
                                start=(k == 0), stop=(k == NKC - 1),
                            )
                            nc.tensor.matmul(
                                pg[:], wts[fc][:, k, FC:2 * FC], xt[:, k, :],
                                start=(k == 0), stop=(k == NKC - 1),
                            )
                    else:
                        for k in range(NKC):
                            nc.tensor.matmul(
                                ph[:], wts[fc][:, k, 0:FC], xt[:, k, :],
                                start=(k == 0), stop=(k == NKC - 1),
                            )
                        for k in range(NKC):
                            nc.tensor.matmul(
                                pg[:], wts[fc][:, k, FC:2 * FC], xt[:, k, :],
                                start=(k == 0), stop=(k == NKC - 1),
                            )
                    st = epool.tile([FC, width], f16, tag="s")
                    ct = epool.tile([FC, width], f16, tag="c")
                    gt = epool.tile([FC, width], f16, tag="g")
                    vt = epool.tile([FC, width], f16, tag="v")
                    # st first: it heads the DVE critical chain (s->g->v->scan)
                    nc.scalar.activation(st[:], ph[:], AF.Sigmoid)
                    nc.scalar.activation(ct[:], pg[:], AF.Sigmoid, scale=-1.0)
                    # g = (hidden + 0.5) max sigmoid(hidden)
                    nc.vector.scalar_tensor_tensor(
                        gt[:], ph[:], 0.5, st[:], op0=OP.add, op1=OP.max
                    )
                    # v = (c - 1)*g = -z*g; the scan subtracts it:
                    #   h = c*h_prev - v = c*h_prev + z*g
                    nc.vector.scalar_tensor_tensor(
                        vt[:], ct[:], 1.0, gt[:], op0=OP.subtract, op1=OP.mult
                    )
                    ho = hall[:, fc, :]
                    pw = WIDTHS[sc - 1]
                    init = 0.0 if sc == 0 else hprev[:, fc, pw - 1:pw]
                    if last and fc == NFC - 1:
                        # The very last feature chunk is the serial tail:
                        # split its scan so the first half's out-DMA overlaps
                        # the second half.
                        hw_ = width // 2
                        nc.vector.tensor_tensor_scan(
                            ho[:, 0:hw_], ct[:, 0:hw_], vt[:, 0:hw_], init,
                            op0=OP.mult, op1=OP.subtract,
                        )
                        nc.sync.dma_start(
                            outT[fc * FC:(fc + 1) * FC, off:off + hw_],
                            ho[:, 0:hw_],
                        )
                        nc.vector.tensor_tensor_scan(
                            ho[:, hw_:width], ct[:, hw_:width], vt[:, hw_:width],
                            hall[:, fc, hw_ - 1:hw_], op0=OP.mult, op1=OP.subtract,
                        )
                        nc.sync.dma_start(
                            outT[fc * FC:(fc + 1) * FC, off + hw_:off + width],
                            ho[:, hw_:width],
                        )
                    elif last:
                        nc.vector.tensor_tensor_scan(
                            ho[:], ct[:], vt[:], init, op0=OP.mult, op1=OP.subtract
                        )
                        nc.sync.dma_start(
                            outT[fc * FC:(fc + 1) * FC, off:off + width], ho[:]
                        )
                    else:
                        nc.vector.tensor_tensor_scan(
                            ho[:], ct[:], vt[:], init, op0=OP.mult, op1=OP.subtract
                        )
                if not last:
                    # Out rides the ACT ring (idle once W lands) so x chunks
                    # never queue behind output traffic on the SP ring.
                    nc.scalar.dma_start(
                        outT[:, off:off + width].rearrange(
                            "(f p) s -> p f s", p=FC
                        ),
                        hall[:],
                    )
                hprev = hall
                off += width

    nc.compile()
    return nc


def _prep_in_maps(x: np.ndarray, W_hg: np.ndarray):
    x = np.asarray(x, dtype=np.float32)
    W_hg = np.asarray(W_hg, dtype=np.float32)
    xTs = [np.ascontiguousarray(x[b].T).astype(np.float16) for b in range(B)]
    wTs = []
    for c in range(2):
        # [D, NFC, 2*FC]: per fc, 128 hidden cols then 128 gate cols
        wt = np.empty((D, NFC, 2 * FC), dtype=np.float32)
        for fc in range(NFC):
            rows_h = W_hg[c * DH + fc * FC:c * DH + (fc + 1) * FC]      # [FC, D]
            rows_g = W_hg[D + c * DH + fc * FC:D + c * DH + (fc + 1) * FC]
            wt[:, fc, 0:FC] = rows_h.T
            wt[:, fc, FC:2 * FC] = rows_g.T
        wTs.append(wt.astype(np.float16))
    return [{"xT": xTs[core // 2], "wT": wTs[core % 2]} for core in range(N_CORES)]


def _get_runner():
    """Build the Bass module once and cache a compiled jax callable for it.

    Mirrors bass2jax.run_bass_via_pjrt's multi-core path, but keeps the
    jitted/sharded executable so repeat kernel() calls skip re-tracing.
    """
    if "runner" in _CACHE:
        return _CACHE["runner"]

    import jax
    from jax.experimental.shard_map import shard_map
    from jax.sharding import Mesh, PartitionSpec
    from concourse import bass2jax

    if "nc" not in _CACHE:
        _CACHE["nc"] = _build()
    nc = _CACHE["nc"]
    bass2jax.install_neuronx_cc_hook()

    in_names = ["xT", "wT"]
    out_name = "outT"
    out_shape, out_dtype = (DH, S), np.float16
    partition_name = nc.partition_id_tensor.name if nc.partition_id_tensor else None

    def _body(xT, wT, zout):
        operands = [xT, wT, zout]
        if partition_name is not None:
            operands.append(bass2jax.partition_id_tensor())
        outs = bass2jax._bass_exec_p.bind(
            *operands,
            out_avals=(jax.core.ShapedArray(out_shape, out_dtype),),
            in_names=tuple(in_names + [out_name] + ([partition_name] if partition_name else [])),
            out_names=(out_name,),
            lowering_input_output_aliases=(),
            sim_require_finite=True,
            sim_require_nnan=True,
            nc=nc,
        )
        return tuple(outs)

    devices = jax.devices()[:N_CORES]
    mesh = Mesh(np.asarray(devices), ("core",))
    sharded = jax.jit(
        shard_map(
            _body, mesh=mesh,
            in_specs=(PartitionSpec("core"),) * 3,
            out_specs=(PartitionSpec("core"),),
            check_rep=False,
        ),
        donate_argnums=(2,),
        keep_unused=True,
    )

    def run(in_maps):
        concat_x = np.concatenate([m["xT"] for m in in_maps], axis=0)
        concat_w = np.concatenate([m["wT"] for m in in_maps], axis=0)
        zeros = np.zeros((N_CORES * DH, S), np.float16)
        (out_arr,) = sharded(concat_x, concat_w, zeros)
        return np.asarray(out_arr).reshape(N_CORES, DH, S)

    _CACHE["runner"] = run
    return run


def kernel(x: np.ndarray, W_hg: np.ndarray) -> np.ndarray:
    run = _get_runner()
    in_maps = _prep_in_maps(x, W_hg)
    outs = run(in_maps)

    out = np.empty((B, S, D), dtype=np.float32)
    for core in range(N_CORES):
        b, c = core // 2, core % 2
        out[b, :, c * DH:(c + 1) * DH] = outs[core].T.astype(np.float32)
    return out


# revision 14
# speedup vs baseline: 1.0412x; 1.0049x over previous
"""MinGRU forward on 8 TRN2 NeuronCores.

Math (linear-space reformulation of the reference's log-space Heinsen scan):
    hg = x @ W_hg.T                       # [B,S,2D]
    hidden, gate = split(hg)
    z = sigmoid(gate)
    c = sigmoid(-gate)                    # = 1 - z
    g = max(hidden + 0.5, sigmoid(hidden))  # == where(h>=0, h+0.5, sigmoid(h))
    u = z * g
    h[t] = c[t] * h[t-1] + u[t]           # convex combination -> bounded, stable
    out = h

Sharding: 8 cores = 4 batches x 2 feature-halves (512 features each).
No cross-core communication: the scan is per-feature independent.

All HBM I/O and matmul operands are fp16 (11 mantissa bits ~ fp32r's
host-rounded 11; verified max rel err ~2.4e-3 vs the 2e-2 gate) which halves
DMA traffic, SBUF footprint and LDWEIGHTS time vs fp32r at the same PE rate.
PSUM accumulation stays fp32; the scan's internal state is fp32.

Engine split per [128, width] tile:
  ACT:  st = sigmoid(ph), ct = sigmoid(-pg)            (2 ops, psum -> fp16)
  DVE:  gt = (ph + 0.5) max st ; vt = (ct - 1)*gt ;
        scan: h = ct*h_prev - vt  (= ct*h_prev + (1-ct)*gt)
  PE:   16 fp16 matmuls (8 ph + 8 pg), psum fp32
  ACT ring: most of W (fc0 k-sliced for a fast first matmul), h out
  SP ring:  x in (chunk 0 k-sliced), fc1-h2/fc2 of W, last-chunk h out
"""

import numpy as np

B, S, D = 4, 4096, 1024
DH = D // 2          # features per core
N_CORES = 8
KC = 128             # contraction chunk
NKC = D // KC        # 8 k chunks
FC = 128             # feature chunk (psum partitions)
NFC = DH // FC       # 4 feature chunks
WIDTHS = [512, 512, 512, 512, 512, 512, 512, 256, 256]
assert sum(WIDTHS) == S

_CACHE = {}


def _build():
    import concourse.bacc as bacc
    import concourse.tile as tile
    import concourse.mybir as mybir

    f32 = mybir.dt.float32
    f16 = mybir.dt.float16
    AF = mybir.ActivationFunctionType
    OP = mybir.AluOpType

    nc = bacc.Bacc("TRN2")
    xT = nc.dram_tensor("xT", [D, S], f16, kind="ExternalInput")
    # wT layout: [D, NFC, 2*FC] — per feature-chunk fc, 128 hidden cols then
    # 128 gate cols.
    wT = nc.dram_tensor("wT", [D, NFC, 2 * FC], f16, kind="ExternalInput")
    outT = nc.dram_tensor("outT", [DH, S], f16, kind="ExternalOutput")

    with tile.TileContext(nc) as tc:
        with (
            tc.tile_pool(name="w", bufs=1) as wpool,
            tc.tile_pool(name="x", bufs=3) as xpool,
            tc.tile_pool(name="ew", bufs=3) as epool,
            tc.tile_pool(name="h", bufs=2) as hpool,
            tc.tile_pool(name="ps", bufs=4, space="PSUM") as pspool,
        ):
            # All W rides the ACT ring; x and out ride the SP ring. fc0 is
            # k-sliced (finest first) so the very first matmul only waits on
            # 64KB, and the fc-major consumption order matches W's arrival
            # order.
            wts = []
            for fc in range(NFC):
                wtf = wpool.tile([KC, NKC, 2 * FC], f16, tag=f"w{fc}")
                wts.append(wtf)

            def load_w(fc, k0, k1):
                nc.scalar.dma_start(
                    wts[fc][:, k0:k1, :],
                    wT[k0 * KC:k1 * KC, fc, :].rearrange("(k p) e -> p k e", p=KC),
                )

            # fc0-k0 split h/g so the first matmul waits on only 32KB.  Few,
            # mostly-coarse issues per ring: each ring has only 8 completion
            # sems, and a 9th in-flight DMA stalls the issuing sequencer on
            # sem rotation.  fc2 rides the SP ring (after x chunk 0) so all
            # of W is resident by the time the PE clock ramp completes.
            nc.scalar.dma_start(wts[0][:, 0, 0:FC], wT[0:KC, 0, 0:FC])
            nc.scalar.dma_start(wts[0][:, 0, FC:2 * FC], wT[0:KC, 0, FC:2 * FC])
            for k0, k1 in [(1, 2), (2, 4), (4, 8)]:
                load_w(0, k0, k1)
            load_w(1, 0, 4)
            load_w(3, 0, 8)

            # x chunk 0 arrives in k-order to pace the first fc's matmuls.
            xt0 = xpool.tile([KC, NKC, WIDTHS[0]], f16, tag="xt")
            xT_r0 = xT[:, 0:WIDTHS[0]].rearrange("(k p) s -> p k s", p=KC)
            for k0, k1 in [(0, 1), (1, 2), (2, 4), (4, 6), (6, 8)]:
                nc.sync.dma_start(xt0[:, k0:k1, :], xT_r0[:, k0:k1, :])
            load_w_sync = lambda fc, k0, k1: nc.sync.dma_start(
                wts[fc][:, k0:k1, :],
                wT[k0 * KC:k1 * KC, fc, :].rearrange("(k p) e -> p k e", p=KC),
            )
            load_w_sync(1, 4, 8)
            load_w_sync(2, 0, 8)

            hprev = None
            off = 0
            for sc, width in enumerate(WIDTHS):
                last = sc == len(WIDTHS) - 1
                if sc == 0:
                    xt = xt0
                else:
                    xt = xpool.tile([KC, NKC, width], f16, tag="xt")
                    xT_r = xT[:, off:off + width].rearrange("(k p) s -> p k s", p=KC)
                    nc.sync.dma_start(xt[:], xT_r)
                hall = hpool.tile([FC, NFC, width], f16, tag="hall")
                for fc in range(NFC):
                    ph = pspool.tile([FC, width], f32, tag="ph")
                    pg = pspool.tile([FC, width], f32, tag="pg")
                    if sc == 0:
                        # Interleave ph/pg per k so each arriving W/x slice
                        # feeds 2 matmuls immediately — keeps the PE fed
                        # while W streams in.
                        for k in range(NKC):
                            nc.tensor.matmul(
                                ph[:], wts[fc][:, k, 0:FC], xt[:, k, :],
                                start=(k == 0), stop=(k == NKC - 1),
                            )
                            nc.tensor.matmul(
                                pg[:], wts[fc][:, k, FC:2 * FC], xt[:, k, :],
                                start=(k == 0), stop=(k == NKC - 1),
                            )
                    else:
                        for k in range(NKC):
                            nc.tensor.matmul(
                                ph[:], wts[fc][:, k, 0:FC], xt[:, k, :],
                                start=(k == 0), stop=(k == NKC - 1),
                            )
                        for k in range(NKC):
                            nc.tensor.matmul(
                                pg[:], wts[fc][:, k, FC:2 * FC], xt[:, k, :],
                                start=(k == 0), stop=(k == NKC - 1),
                            )
                    st = epool.tile([FC, width], f16, tag="s")
                    ct = epool.tile([FC, width], f16, tag="c")
                    gt = epool.tile([FC, width], f16, tag="g")
                    vt = epool.tile([FC, width], f16, tag="v")
                    # st first: it heads the DVE critical chain (s->g->v->scan)
                    nc.scalar.activation(st[:], ph[:], AF.Sigmoid)
                    nc.scalar.activation(ct[:], pg[:], AF.Sigmoid, scale=-1.0)
                    # g = (hidden + 0.5) max sigmoid(hidden)
                    nc.vector.scalar_tensor_tensor(
                        gt[:], ph[:], 0.5, st[:], op0=OP.add, op1=OP.max
                    )
                    # v = (c - 1)*g = -z*g; the scan subtracts it:
                    #   h = c*h_prev - v = c*h_prev + z*g
                    nc.vector.scalar_tensor_tensor(
                        vt[:], ct[:], 1.0, gt[:], op0=OP.subtract, op1=OP.mult
                    )
                    ho = hall[:, fc, :]
                    pw = WIDTHS[sc - 1]
                    init = 0.0 if sc == 0 else hprev[:, fc, pw - 1:pw]
                    if last and fc == NFC - 1:
                        # The very last feature chunk is the serial tail:
                        # split its scan so the first half's out-DMA overlaps
                        # the second half.
                        hw_ = width // 2
                        nc.vector.tensor_tensor_scan(
                            ho[:, 0:hw_], ct[:, 0:hw_], vt[:, 0:hw_], init,
                            op0=OP.mult, op1=OP.subtract,
                        )
                        nc.sync.dma_start(
                            outT[fc * FC:(fc + 1) * FC, off:off + hw_],
                            ho[:, 0:hw_],
                        )
                        nc.vector.tensor_tensor_scan(
                            ho[:, hw_:width], ct[:, hw_:width], vt[:, hw_:width],
                            hall[:, fc, hw_ - 1:hw_], op0=OP.mult, op1=OP.subtract,
                        )
                        nc.sync.dma_start(
                            outT[fc * FC:(fc + 1) * FC, off + hw_:off + width],
                            ho[:, hw_:width],
                        )
                    elif last:
                        nc.vector.tensor_tensor_scan(
                            ho[:], ct[:], vt[:], init, op0=OP.mult, op1=OP.subtract
                        )
                        nc.sync.dma_start(
                            outT[fc * FC:(fc + 1) * FC, off:off + width], ho[:]
                        )
                    else:
                        nc.vector.tensor_tensor_scan(
                            ho[:], ct[:], vt[:], init, op0=OP.mult, op1=OP.subtract
                        )
                if not last:
                    # Out rides the ACT ring (idle once W lands) so x chunks
                    # never queue behind output traffic on the SP ring.
                    nc.scalar.dma_start(
                        outT[:, off:off + width].rearrange(
                            "(f p) s -> p f s", p=FC
                        ),
                        hall[:],
                    )
                hprev = hall
                off += width

    nc.compile()
    return nc


def _prep_in_maps(x: np.ndarray, W_hg: np.ndarray):
    x = np.asarray(x, dtype=np.float32)
    W_hg = np.asarray(W_hg, dtype=np.float32)
    xTs = [np.ascontiguousarray(x[b].T).astype(np.float16) for b in range(B)]
    wTs = []
    for c in range(2):
        # [D, NFC, 2*FC]: per fc, 128 hidden cols then 128 gate cols
        wt = np.empty((D, NFC, 2 * FC), dtype=np.float32)
        for fc in range(NFC):
            rows_h = W_hg[c * DH + fc * FC:c * DH + (fc + 1) * FC]      # [FC, D]
            rows_g = W_hg[D + c * DH + fc * FC:D + c * DH + (fc + 1) * FC]
            wt[:, fc, 0:FC] = rows_h.T
            wt[:, fc, FC:2 * FC] = rows_g.T
        wTs.append(wt.astype(np.float16))
    return [{"xT": xTs[core // 2], "wT": wTs[core % 2]} for core in range(N_CORES)]


def _get_runner():
    """Build the Bass module once and cache a compiled jax callable for it.

    Mirrors bass2jax.run_bass_via_pjrt's multi-core path, but keeps the
    jitted/sharded executable so repeat kernel() calls skip re-tracing.
    """
    if "runner" in _CACHE:
        return _CACHE["runner"]

    import jax
    from jax.experimental.shard_map import shard_map
    from jax.sharding import Mesh, PartitionSpec
    from concourse import bass2jax

    if "nc" not in _CACHE:
        _CACHE["nc"] = _build()
    nc = _CACHE["nc"]
    bass2jax.install_neuronx_cc_hook()

    in_names = ["xT", "wT"]
    out_name = "outT"
    out_shape, out_dtype = (DH, S), np.float16
    partition_name = nc.partition_id_tensor.name if nc.partition_id_tensor else None

    def _body(xT, wT, zout):
        operands = [xT, wT, zout]
        if partition_name is not None:
            operands.append(bass2jax.partition_id_tensor())
        outs = bass2jax._bass_exec_p.bind(
            *operands,
            out_avals=(jax.core.ShapedArray(out_shape, out_dtype),),
            in_names=tuple(in_names + [out_name] + ([partition_name] if partition_name else [])),
            out_names=(out_name,),
            lowering_input_output_aliases=(),
            sim_require_finite=True,
            sim_require_nnan=True,
            nc=nc,
        )
        return tuple(outs)

    devices = jax.devices()[:N_CORES]
    mesh = Mesh(np.asarray(devices), ("core",))
    sharded = jax.jit(
        shard_map(
            _body, mesh=mesh,
            in_specs=(PartitionSpec("core"),) * 3,
            out_specs=(PartitionSpec("core"),),
            check_rep=False,
        ),
        donate_argnums=(2,),
        keep_unused=True,
    )

    def run(in_maps):
        concat_x = np.concatenate([m["xT"] for m in in_maps], axis=0)
        concat_w = np.concatenate([m["wT"] for m in in_maps], axis=0)
        zeros = np.zeros((N_CORES * DH, S), np.float16)
        (out_arr,) = sharded(concat_x, concat_w, zeros)
        return np.asarray(out_arr).reshape(N_CORES, DH, S)

    _CACHE["runner"] = run
    return run


def kernel(x: np.ndarray, W_hg: np.ndarray) -> np.ndarray:
    run = _get_runner()
    in_maps = _prep_in_maps(x, W_hg)
    outs = run(in_maps)

    out = np.empty((B, S, D), dtype=np.float32)
    for core in range(N_CORES):
        b, c = core // 2, core % 2
        out[b, :, c * DH:(c + 1) * DH] = outs[core].T.astype(np.float32)
    return out
